# revision 1
# baseline (speedup 1.0000x reference)
"""ArcFace (AngularPenaltySMLoss) forward on 8 TRN2 NeuronCores.

loss = -mean_i( num_i - log(exp(num_i) + sum_j exp(S*wf[i,j]) - exp(S*wf[i,y_i])) )
  with num_i = S*cos(acos(clip(wf[i,y_i])) + M) = S*(cosM*t - sinM*sqrt(1-t^2))

Sharding: data-parallel over the batch dim (1024 rows per core). Each core
streams its [1024, 10000] f32 shard through SBUF in 8 tiles of 128 rows,
ScalarE computes exp(S*x) with a fused per-row accumulate (accum_out), the
per-row target logits are fetched with a gpsimd indirect DMA, and a small
epilogue computes the per-row loss terms and a per-partition partial sum.
Host sums the 8x128 partials (the gather/unshard step).
"""

import math
import os
import sys

import numpy as np

B, C = 8192, 10000
NCORES = 8
B_LOC = B // NCORES  # 1024
P = 128
T = B_LOC // P  # 8 row-tiles per core; row r = p*T + t maps to [p, t]
S = 64.0
MARGIN = 0.5
EPS = 1e-7
LASTCH = 4      # column chunks for the LAST row-tile only (C must divide)

LAST_EXEC_NS = None
LAST_RESULTS = None


def _import_concourse():
    try:
        import concourse  # noqa: F401
    except ImportError:
        sys.path.insert(0, "/opt/trn_rl_repo")


def _build_nc(stage="full"):
    """stage: 'prologue' (gather only), 'mainloop' (+exp/rowsum), 'full',
    or 'full:<subnum>' to truncate the epilogue after N ops."""
    stage_sub = 99
    if stage.startswith("full:"):
        stage, stage_sub = "full", int(stage.split(":")[1])
    _import_concourse()
    import concourse.bass as bass
    import concourse.tile as tile
    from concourse import bacc, mybir

    f32 = mybir.dt.float32
    i32 = mybir.dt.int32
    AF = mybir.ActivationFunctionType
    OP = mybir.AluOpType

    COSM = math.cos(MARGIN)
    SINM = math.sin(MARGIN)

    nc = bacc.Bacc()
    wf_ext = nc.declare_dram_parameter("wf", [B_LOC, C], f32, isOutput=False)
    labels_ext = nc.declare_dram_parameter("labels", [B_LOC], i32, isOutput=False)
    out_ext = nc.declare_dram_parameter("out", [P, 1], f32, isOutput=True)

    # wf rows regrouped so row p*T + t lands on partition p, column t
    wf_by_pt = wf_ext[:, :].rearrange("(p t) c -> p t c", t=T)
    lab_by_pt = labels_ext[:].rearrange("(p t) -> p t", t=T)
    # flat [B_LOC*C] element view of the shard, for the indirect gather
    wf_flat = bass.AP(tensor=wf_ext, offset=0, ap=[[1, B_LOC * C], [1, 1]])

    with tile.TileContext(nc) as tc:
        with (
            tc.tile_pool(name="wfpool", bufs=3) as wfpool,
            tc.tile_pool(name="lastpool", bufs=2) as lastpool,
            tc.tile_pool(name="scratch", bufs=1) as scratch,
            tc.tile_pool(name="small", bufs=1) as small,
        ):
            rowsum = small.tile([P, T], f32)  # per-row sum_j exp(S*wf[r, j])
            last_parts = small.tile([P, LASTCH], f32)  # last tile's chunk sums
            tgt = small.tile([P, T], f32)     # per-row wf[r, labels[r]]
            labels_sb = small.tile([P, T], i32)
            labels_cp = small.tile([P, T], i32)
            flat_idx = small.tile([P, T], i32)

            nc.sync.dma_start(out=labels_sb[:], in_=lab_by_pt)
            # flat_idx[p, t] = (p*T + t)*C + labels[p*T + t]
            # Funnel the two dependencies (iota, labels DMA) through gpsimd
            # program order so no instruction needs more than one sync wait.
            nc.gpsimd.iota(
                flat_idx[:], pattern=[[C, T]], base=0, channel_multiplier=T * C
            )
            nc.gpsimd.tensor_copy(labels_cp[:], labels_sb[:])
            nc.gpsimd.tensor_add(flat_idx[:], flat_idx[:], labels_cp[:])

            # one indirect DMA per column: multi-index-per-partition offset APs
            # compute bogus addresses on real HW (sim accepts them), so stick
            # to the proven [P, 1] single-index-per-partition form
            for t in range(T):
                nc.gpsimd.indirect_dma_start(
                    out=tgt[:, t : t + 1],
                    out_offset=None,
                    in_=wf_flat,
                    in_offset=bass.IndirectOffsetOnAxis(
                        ap=flat_idx[:, t : t + 1], axis=0
                    ),
                )

            if stage == "prologue":
                nc.sync.dma_start(out=out_ext[:, :], in_=tgt[:, 0:1])

            # main pass: exp(S*wf) with fused per-row accumulation. Tiles
            # 0..T-2 use full-width DMAs (big transfers = best HBM efficiency);
            # the last tile is chunked along the class dim so only ~1/LASTCH
            # of its exp work remains after the final DMA byte lands.
            W = C // LASTCH
            if stage != "prologue":
                for t in range(T - 1):
                    wf_tile = wfpool.tile([P, C], f32, tag="wf_full")
                    nc.sync.dma_start(out=wf_tile[:], in_=wf_by_pt[:, t, :])
                    e_scr = scratch.tile([P, C], f32, tag="esc")
                    nc.scalar.activation(
                        out=e_scr[:],
                        in_=wf_tile[:],
                        func=AF.Exp,
                        scale=S,
                        accum_out=rowsum[:, t : t + 1],
                    )
                for j in range(LASTCH):
                    wf_ck = lastpool.tile([P, W], f32, tag="wf_last")
                    nc.sync.dma_start(
                        out=wf_ck[:],
                        in_=wf_by_pt[:, T - 1, j * W : (j + 1) * W],
                    )
                    e_scr = scratch.tile([P, C], f32, tag="esc")
                    nc.scalar.activation(
                        out=e_scr[:, :W],
                        in_=wf_ck[:],
                        func=AF.Exp,
                        scale=S,
                        accum_out=last_parts[:, j : j + 1],
                    )
                nc.vector.tensor_reduce(
                    out=rowsum[:, T - 1 : T], in_=last_parts[:],
                    axis=mybir.AxisListType.X, op=OP.add,
                )

            if stage == "mainloop":
                nc.sync.dma_start(out=out_ext[:, :], in_=rowsum[:, 0:1])

            if stage != "full":
                pass
            else:
                run_epilogue(
                    nc, bass, tile, mybir, small,
                    rowsum, tgt, out_ext, COSM, SINM, stage_sub,
                )

    nc.compile()
    _force_single_act_table(nc)
    return nc


def _force_single_act_table(nc, set_id=6):
    """All ACT functions used here (Exp, Ln, Square) live together in set 6
    (natural_log_exp_and_others), but the table-load pass greedily picks the
    first set per function (exp_and_others / natural_log), inserting four
    table loads -- one of them right on the critical tail before the final
    Ln. Point the first load at set 6 and drop the now-redundant rest."""
    from concourse import mybir

    for blk in nc.main_func.blocks:
        il = blk.instructions
        loads = [i for i in il if isinstance(i, mybir.InstLoadActFuncSet)]
        if not loads:
            continue
        for inst in loads:
            si = inst.sync_info
            assert si is None or (not si.on_wait and not si.on_update), (
                "table load carries sync; refusing to drop it"
            )
            inst.act_func_set_id = set_id
        first = loads[0]
        blk.instructions = [
            i
            for i in il
            if not (isinstance(i, mybir.InstLoadActFuncSet) and i is not first)
        ]


def run_epilogue(nc, bass, tile, mybir, small, rowsum, tgt, out_ext, COSM,
                 SINM, sub=99):
    f32 = mybir.dt.float32
    AF = mybir.ActivationFunctionType
    OP = mybir.AluOpType

    steps = [0]

    def cut(buf):
        steps[0] += 1
        if steps[0] == sub:
            nc.sync.dma_start(out=out_ext[:, :], in_=buf[:, 0:1])
            return True
        return False
    # epilogue on [P, T] tensors
    t_clip = small.tile([P, T], f32)
    tsq = small.tile([P, T], f32)
    omt = small.tile([P, T], f32)
    lnomt = small.tile([P, T], f32)
    sq_sin = small.tile([P, T], f32)
    bterm = small.tile([P, T], f32)
    num = small.tile([P, T], f32)
    e_num = small.tile([P, T], f32)
    e_tgt = small.tile([P, T], f32)
    den = small.tile([P, T], f32)
    lnden = small.tile([P, T], f32)
    lbuf = small.tile([P, T], f32)
    partial = small.tile([P, 1], f32)

    nc.vector.tensor_scalar(
        out=t_clip[:], in0=tgt[:],
        scalar1=-1.0 + EPS, scalar2=1.0 - EPS, op0=OP.max, op1=OP.min,
    )
    if cut(t_clip):
        return
    nc.scalar.activation(out=tsq[:], in_=t_clip[:], func=AF.Square)
    if cut(tsq):
        return
    nc.vector.tensor_scalar(
        out=omt[:], in0=tsq[:],
        scalar1=-1.0, scalar2=1.0, op0=OP.mult, op1=OP.add,
    )
    if cut(omt):
        return
    # sqrt(1-t^2) = exp(0.5*ln(1-t^2)); keeps Ln/Exp in one ACT table set
    nc.scalar.activation(out=lnomt[:], in_=omt[:], func=AF.Ln)
    if cut(lnomt):
        return
    nc.scalar.activation(out=sq_sin[:], in_=lnomt[:], func=AF.Exp, scale=0.5)
    if cut(sq_sin):
        return
    nc.vector.tensor_scalar_mul(out=bterm[:], in0=sq_sin[:], scalar1=S * SINM)
    if cut(bterm):
        return
    nc.vector.scalar_tensor_tensor(
        out=num[:], in0=t_clip[:], scalar=S * COSM, in1=bterm[:],
        op0=OP.mult, op1=OP.subtract,
    )
    if cut(num):
        return
    nc.scalar.activation(out=e_num[:], in_=num[:], func=AF.Exp)
    if cut(e_num):
        return
    nc.scalar.activation(out=e_tgt[:], in_=tgt[:], func=AF.Exp, scale=S)
    if cut(e_tgt):
        return
    # d0 = e_num - e_tgt depends only on tgt, so the scheduler hoists it off
    # the critical tail; den needs a single add once rowsum lands
    d0 = small.tile([P, T], f32)
    nc.vector.tensor_sub(out=d0[:], in0=e_num[:], in1=e_tgt[:])
    nc.vector.tensor_add(out=den[:], in0=rowsum[:], in1=d0[:])
    if cut(den):
        return
    # denominator reaches ~1e31 but the ScalarE ln LUT only covers
    # [-2^64, 2^64]; compute ln(den * 2^-40) + 40*ln2 instead, folding
    # the +40*ln2 per-element constant into the reduction's initial
    # value (T elements per partition => -T*40*ln2).
    LNSHIFT = 40
    nc.scalar.activation(
        out=lnden[:], in_=den[:], func=AF.Ln, scale=float(2.0**-LNSHIFT)
    )
    if cut(lnden):
        return
    # num_adj = num - LNSHIFT*ln2 is hoistable (depends only on tgt); the
    # per-element constant compensates the scaled ln, so no final scalar add
    num_adj = small.tile([P, T], f32)
    nc.vector.tensor_scalar_add(
        out=num_adj[:], in0=num[:], scalar1=float(-LNSHIFT * math.log(2.0))
    )
    nc.vector.tensor_sub(out=lbuf[:], in0=num_adj[:], in1=lnden[:])
    nc.vector.tensor_reduce(
        out=partial[:], in_=lbuf[:], axis=mybir.AxisListType.X, op=OP.add
    )
    # the sync HWDGE ring is backed up with the 29 wf-load completions at
    # kernel end; issue the tiny output DMA on the idle gpsimd SWDGE queue so
    # its completion semaphore (which the exit drain waits on) fires sooner
    nc.gpsimd.dma_start(out=out_ext[:, :], in_=partial[:])


def kernel(**inputs) -> np.ndarray:
    global LAST_EXEC_NS, LAST_RESULTS
    _import_concourse()
    from concourse.bass_utils import run_bass_kernel_spmd

    wf = np.asarray(inputs["wf"], dtype=np.float32)
    labels = np.asarray(inputs["labels"]).astype(np.int32)

    in_maps = []
    for c in range(NCORES):
        sl = slice(c * B_LOC, (c + 1) * B_LOC)
        in_maps.append(
            {
                "wf": np.ascontiguousarray(wf[sl]),
                "labels": np.ascontiguousarray(labels[sl]),
            }
        )

    nc = _build_nc()
    trace = os.environ.get("KERNEL_TRACE", "0") == "1"
    res = run_bass_kernel_spmd(
        nc, in_maps, core_ids=list(range(NCORES)), trace=trace
    )
    LAST_EXEC_NS = res.exec_time_ns
    LAST_RESULTS = res

    total = 0.0
    for r in res.results:
        total += float(r["out"].astype(np.float64).sum())
    return np.asarray(np.float32(-(total / B)))


if __name__ == "__main__":
    rng = np.random.default_rng(0)
    wf = rng.random((B, C), dtype=np.float32)
    labels = rng.integers(0, C, size=(B,)).astype(np.int64)
    print(kernel(wf=wf, labels=labels))



# revision 3
# speedup vs baseline: 1.0389x; 1.0389x over previous
"""ArcFace (AngularPenaltySMLoss) forward on 8 TRN2 NeuronCores.

loss = -mean_i( num_i - log(exp(num_i) + sum_j exp(S*wf[i,j]) - exp(S*wf[i,y_i])) )
  with num_i = S*cos(acos(clip(wf[i,y_i])) + M) = S*(cosM*t - sinM*sqrt(1-t^2))

Sharding: data-parallel over the batch dim (1024 rows per core). Each core
streams its [1024, 10000] f32 shard through SBUF in 8 tiles of 128 rows,
ScalarE computes exp(S*x) with a fused per-row accumulate (accum_out), the
per-row target logits are fetched with a gpsimd indirect DMA, and a small
epilogue computes the per-row loss terms and a per-partition partial sum.
Host sums the 8x128 partials (the gather/unshard step).
"""

import math
import os
import sys

import numpy as np

B, C = 8192, 10000
NCORES = 8
B_LOC = B // NCORES  # 1024
P = 128
T = B_LOC // P  # 8 row-tiles per core; row r = p*T + t maps to [p, t]
S = 64.0
MARGIN = 0.5
EPS = 1e-7
LASTCH = 4      # column chunks for the LAST row-tile only (C must divide)

LAST_EXEC_NS = None
LAST_RESULTS = None


def _import_concourse():
    try:
        import concourse  # noqa: F401
    except ImportError:
        sys.path.insert(0, "/opt/trn_rl_repo")


def _build_nc(stage="full"):
    """stage: 'prologue' (gather only), 'mainloop' (+exp/rowsum), 'full',
    or 'full:<subnum>' to truncate the epilogue after N ops."""
    stage_sub = 99
    if stage.startswith("full:"):
        stage, stage_sub = "full", int(stage.split(":")[1])
    _import_concourse()
    import concourse.bass as bass
    import concourse.tile as tile
    from concourse import bacc, mybir

    f32 = mybir.dt.float32
    i32 = mybir.dt.int32
    AF = mybir.ActivationFunctionType
    OP = mybir.AluOpType

    COSM = math.cos(MARGIN)
    SINM = math.sin(MARGIN)

    nc = bacc.Bacc()
    wf_ext = nc.declare_dram_parameter("wf", [B_LOC, C], f32, isOutput=False)
    labels_ext = nc.declare_dram_parameter("labels", [B_LOC], i32, isOutput=False)
    out_ext = nc.declare_dram_parameter("out", [P, 1], f32, isOutput=True)

    # wf rows regrouped so row p*T + t lands on partition p, column t
    wf_by_pt = wf_ext[:, :].rearrange("(p t) c -> p t c", t=T)
    lab_by_pt = labels_ext[:].rearrange("(p t) -> p t", t=T)
    # flat [B_LOC*C] element view of the shard, for the indirect gather
    wf_flat = bass.AP(tensor=wf_ext, offset=0, ap=[[1, B_LOC * C], [1, 1]])

    with tile.TileContext(nc) as tc:
        with (
            tc.tile_pool(name="wfpool", bufs=3) as wfpool,
            tc.tile_pool(name="lastpool", bufs=2) as lastpool,
            tc.tile_pool(name="scratch", bufs=1) as scratch,
            tc.tile_pool(name="small", bufs=1) as small,
        ):
            rowsum = small.tile([P, T], f32)  # per-row sum_j exp(S*wf[r, j])
            last_parts = small.tile([P, LASTCH], f32)  # last tile's chunk sums
            tgt = small.tile([P, T], f32)     # per-row wf[r, labels[r]]
            labels_sb = small.tile([P, T], i32)
            labels_cp = small.tile([P, T], i32)
            flat_idx = small.tile([P, T], i32)

            nc.sync.dma_start(out=labels_sb[:], in_=lab_by_pt)
            # flat_idx[p, t] = (p*T + t)*C + labels[p*T + t]
            # Funnel the two dependencies (iota, labels DMA) through gpsimd
            # program order so no instruction needs more than one sync wait.
            nc.gpsimd.iota(
                flat_idx[:], pattern=[[C, T]], base=0, channel_multiplier=T * C
            )
            nc.gpsimd.tensor_copy(labels_cp[:], labels_sb[:])
            nc.gpsimd.tensor_add(flat_idx[:], flat_idx[:], labels_cp[:])

            # one indirect DMA per column: multi-index-per-partition offset APs
            # compute bogus addresses on real HW (sim accepts them), so stick
            # to the proven [P, 1] single-index-per-partition form
            for t in range(T):
                nc.gpsimd.indirect_dma_start(
                    out=tgt[:, t : t + 1],
                    out_offset=None,
                    in_=wf_flat,
                    in_offset=bass.IndirectOffsetOnAxis(
                        ap=flat_idx[:, t : t + 1], axis=0
                    ),
                )

            if stage == "prologue":
                nc.sync.dma_start(out=out_ext[:, :], in_=tgt[:, 0:1])

            # main pass: exp(S*wf) with fused per-row accumulation. Tiles
            # 0..T-2 use full-width DMAs (big transfers = best HBM efficiency);
            # the last tile is chunked along the class dim so only ~1/LASTCH
            # of its exp work remains after the final DMA byte lands.
            W = C // LASTCH
            if stage != "prologue":
                for t in range(T - 1):
                    wf_tile = wfpool.tile([P, C], f32, tag="wf_full")
                    nc.sync.dma_start(out=wf_tile[:], in_=wf_by_pt[:, t, :])
                    e_scr = scratch.tile([P, C], f32, tag="esc")
                    nc.scalar.activation(
                        out=e_scr[:],
                        in_=wf_tile[:],
                        func=AF.Exp,
                        scale=S,
                        accum_out=rowsum[:, t : t + 1],
                    )
                for j in range(LASTCH):
                    wf_ck = lastpool.tile([P, W], f32, tag="wf_last")
                    nc.sync.dma_start(
                        out=wf_ck[:],
                        in_=wf_by_pt[:, T - 1, j * W : (j + 1) * W],
                    )
                    e_scr = scratch.tile([P, C], f32, tag="esc")
                    nc.scalar.activation(
                        out=e_scr[:, :W],
                        in_=wf_ck[:],
                        func=AF.Exp,
                        scale=S,
                        accum_out=last_parts[:, j : j + 1],
                    )
                nc.vector.tensor_reduce(
                    out=rowsum[:, T - 1 : T], in_=last_parts[:],
                    axis=mybir.AxisListType.X, op=OP.add,
                )

            if stage == "mainloop":
                nc.sync.dma_start(out=out_ext[:, :], in_=rowsum[:, 0:1])

            if stage != "full":
                pass
            else:
                run_epilogue(
                    nc, bass, tile, mybir, small,
                    rowsum, tgt, out_ext, COSM, SINM, stage_sub,
                )

    nc.compile()
    _force_single_act_table(nc)
    return nc


def _force_single_act_table(nc, set_id=6):
    """All ACT functions used here (Exp, Ln, Square) live together in set 6
    (natural_log_exp_and_others), but the table-load pass greedily picks the
    first set per function (exp_and_others / natural_log), inserting four
    table loads -- one of them right on the critical tail before the final
    Ln. Point the first load at set 6 and drop the now-redundant rest."""
    from concourse import mybir

    for blk in nc.main_func.blocks:
        il = blk.instructions
        loads = [i for i in il if isinstance(i, mybir.InstLoadActFuncSet)]
        if not loads:
            continue
        for inst in loads:
            si = inst.sync_info
            assert si is None or (not si.on_wait and not si.on_update), (
                "table load carries sync; refusing to drop it"
            )
            inst.act_func_set_id = set_id
        first = loads[0]
        blk.instructions = [
            i
            for i in il
            if not (isinstance(i, mybir.InstLoadActFuncSet) and i is not first)
        ]


def run_epilogue(nc, bass, tile, mybir, small, rowsum, tgt, out_ext, COSM,
                 SINM, sub=99):
    f32 = mybir.dt.float32
    AF = mybir.ActivationFunctionType
    OP = mybir.AluOpType

    steps = [0]

    def cut(buf):
        steps[0] += 1
        if steps[0] == sub:
            nc.sync.dma_start(out=out_ext[:, :], in_=buf[:, 0:1])
            return True
        return False
    # The Tile scheduler's cost model assumes the indirect tgt gather lands
    # quickly, so it fronts the tgt-dependent ACTs (and their gather waits)
    # on the scalar queue ahead of the main-loop EXPs. On hardware the
    # gather's tiny SWDGE packets starve behind the saturated wf stream
    # (~40us), stalling the first big EXP until ~45us and cascading into
    # two long DMA stalls. Gate the whole tgt chain on rowsum[:,2] (a
    # numerically-zero add) so the scheduler must order it after EXP-tile-2.
    gate = small.tile([P, 1], f32)
    tgt2 = small.tile([P, T], f32)
    nc.vector.tensor_scalar_mul(out=gate[:], in0=rowsum[:, 2:3], scalar1=0.0)
    nc.vector.tensor_scalar(
        out=tgt2[:], in0=tgt[:], scalar1=gate[:, 0:1], scalar2=None, op0=OP.add
    )
    tgt = tgt2
    # epilogue on [P, T] tensors
    t_clip = small.tile([P, T], f32)
    tsq = small.tile([P, T], f32)
    omt = small.tile([P, T], f32)
    lnomt = small.tile([P, T], f32)
    sq_sin = small.tile([P, T], f32)
    bterm = small.tile([P, T], f32)
    num = small.tile([P, T], f32)
    e_num = small.tile([P, T], f32)
    e_tgt = small.tile([P, T], f32)
    den = small.tile([P, T], f32)
    lnden = small.tile([P, T], f32)
    lbuf = small.tile([P, T], f32)
    partial = small.tile([P, 1], f32)

    nc.vector.tensor_scalar(
        out=t_clip[:], in0=tgt[:],
        scalar1=-1.0 + EPS, scalar2=1.0 - EPS, op0=OP.max, op1=OP.min,
    )
    if cut(t_clip):
        return
    nc.scalar.activation(out=tsq[:], in_=t_clip[:], func=AF.Square)
    if cut(tsq):
        return
    nc.vector.tensor_scalar(
        out=omt[:], in0=tsq[:],
        scalar1=-1.0, scalar2=1.0, op0=OP.mult, op1=OP.add,
    )
    if cut(omt):
        return
    # sqrt(1-t^2) = exp(0.5*ln(1-t^2)); keeps Ln/Exp in one ACT table set
    nc.scalar.activation(out=lnomt[:], in_=omt[:], func=AF.Ln)
    if cut(lnomt):
        return
    nc.scalar.activation(out=sq_sin[:], in_=lnomt[:], func=AF.Exp, scale=0.5)
    if cut(sq_sin):
        return
    nc.vector.tensor_scalar_mul(out=bterm[:], in0=sq_sin[:], scalar1=S * SINM)
    if cut(bterm):
        return
    nc.vector.scalar_tensor_tensor(
        out=num[:], in0=t_clip[:], scalar=S * COSM, in1=bterm[:],
        op0=OP.mult, op1=OP.subtract,
    )
    if cut(num):
        return
    nc.scalar.activation(out=e_num[:], in_=num[:], func=AF.Exp)
    if cut(e_num):
        return
    nc.scalar.activation(out=e_tgt[:], in_=tgt[:], func=AF.Exp, scale=S)
    if cut(e_tgt):
        return
    # d0 = e_num - e_tgt depends only on tgt, so the scheduler hoists it off
    # the critical tail; den needs a single add once rowsum lands
    d0 = small.tile([P, T], f32)
    nc.vector.tensor_sub(out=d0[:], in0=e_num[:], in1=e_tgt[:])
    nc.vector.tensor_add(out=den[:], in0=rowsum[:], in1=d0[:])
    if cut(den):
        return
    # denominator reaches ~1e31 but the ScalarE ln LUT only covers
    # [-2^64, 2^64]; compute ln(den * 2^-40) + 40*ln2 instead, folding
    # the +40*ln2 per-element constant into the reduction's initial
    # value (T elements per partition => -T*40*ln2).
    LNSHIFT = 40
    nc.scalar.activation(
        out=lnden[:], in_=den[:], func=AF.Ln, scale=float(2.0**-LNSHIFT)
    )
    if cut(lnden):
        return
    # num_adj = num - LNSHIFT*ln2 is hoistable (depends only on tgt); the
    # per-element constant compensates the scaled ln, so no final scalar add
    num_adj = small.tile([P, T], f32)
    nc.vector.tensor_scalar_add(
        out=num_adj[:], in0=num[:], scalar1=float(-LNSHIFT * math.log(2.0))
    )
    nc.vector.tensor_sub(out=lbuf[:], in0=num_adj[:], in1=lnden[:])
    nc.vector.tensor_reduce(
        out=partial[:], in_=lbuf[:], axis=mybir.AxisListType.X, op=OP.add
    )
    # Keep the output DMA off gpsimd: its SWDGE drain costs ~7us, and with
    # no late gpsimd DMA it runs early (right after the gather issues) far
    # off the critical tail. By output time the sync HWDGE ring has retired
    # all wf-load completions, so the sync queue completes it immediately.
    nc.sync.dma_start(out=out_ext[:, :], in_=partial[:])


def kernel(**inputs) -> np.ndarray:
    global LAST_EXEC_NS, LAST_RESULTS
    _import_concourse()
    from concourse.bass_utils import run_bass_kernel_spmd

    wf = np.asarray(inputs["wf"], dtype=np.float32)
    labels = np.asarray(inputs["labels"]).astype(np.int32)

    in_maps = []
    for c in range(NCORES):
        sl = slice(c * B_LOC, (c + 1) * B_LOC)
        in_maps.append(
            {
                "wf": np.ascontiguousarray(wf[sl]),
                "labels": np.ascontiguousarray(labels[sl]),
            }
        )

    nc = _build_nc()
    trace = os.environ.get("KERNEL_TRACE", "0") == "1"
    res = run_bass_kernel_spmd(
        nc, in_maps, core_ids=list(range(NCORES)), trace=trace
    )
    LAST_EXEC_NS = res.exec_time_ns
    LAST_RESULTS = res

    total = 0.0
    for r in res.results:
        total += float(r["out"].astype(np.float64).sum())
    return np.asarray(np.float32(-(total / B)))


if __name__ == "__main__":
    rng = np.random.default_rng(0)
    wf = rng.random((B, C), dtype=np.float32)
    labels = rng.integers(0, C, size=(B,)).astype(np.int64)
    print(kernel(wf=wf, labels=labels))

